# revision 24
# baseline (speedup 1.0000x reference)
"""DeepseekV2 decoder layer (MLA attention + SwiGLU MLP) on 8 TRN2 NeuronCores.

Sharding: core c -> batch b = c//4, j = c%4. The core owns 512 query rows,
packed as two 256-row strips: E = [j*256,(j+1)*256) and L = [(7-j)*256,
(8-j)*256). E+L pairs balance the causal-attention load exactly across the
four cores of a batch: key tiles 0..7 are processed for both strips (the L
strip starts at row >= 1024, so those tiles need no mask there), key tiles
8..15 only for the L strip at half width. Every core computes the
full-sequence KV latents for its batch, its own 512 rows through attention +
o_proj + FFN, and returns its 512 output rows. No collectives.

All cores run one identical SPMD program; per-core position enters only
through input data (causal masks, sliced hidden/rope tables).

On-device layout is feature-major (activations transposed, features on
partitions) so no transposes are ever needed: for y = x @ W the device
computes y^T = matmul(lhsT=W_tile, rhs=x^T_tile) accumulating K-tiles in
PSUM. RMSNorm weights are folded into adjacent weight matrices on the host;
RMSNorm 1/rms column scales are applied when copying matmul outputs from
PSUM to SBUF; cross-partition reductions use ones-vector matmuls.

RoPE is evaluated as out = x*cos + swap(x)*sin_signed where swap exchanges
the two 32-row halves (one DMA per half) and the sign pattern is folded into
the sin table on the host; q heads are roped in pairs so every q_b rope
matmul runs with a full 128-wide stationary operand.
"""

import json

import numpy as np
import ml_dtypes

B, S, H = 2, 2048, 2048
NH = 16
Q_LORA = 1536
KV_LORA = 512
NOPE = 128
ROPE = 64
QHD = NOPE + ROPE  # 192
VHD = 128
FF = 8192
EPS = 1e-6
P = 128
QR = 512  # query rows per core (two 256-row strips)
QH = 256  # strip width
TK = S // P  # 16 key tiles
KI_H = H // P  # 16
KI_QL = Q_LORA // P  # 12
KI_KVL = KV_LORA // P  # 4
NF_FF = FF // P  # 64
ATTN_SCALE = QHD ** -0.5

BF16 = ml_dtypes.bfloat16

_COMPILED = {}


# ---------------------------------------------------------------------------
# compiler workaround: this container's walrus rejects >1 sem wait per
# instruction; split extra waits onto single-wait NoOps.
# ---------------------------------------------------------------------------
def _install_multiwait_fix(bass):
    if getattr(bass.Bass, "_multiwait_fix_installed", False):
        return
    orig = bass.Bass.to_json_bytes

    def _split(m):
        for f in m.get("functions", []):
            for b in f.get("blocks", []):
                out = []
                for inst in b.get("instructions", []):
                    si = inst.get("sync_info") or {}
                    waits = si.get("on_wait") or []
                    if len(waits) > 1:
                        for k, w in enumerate(waits[:-1]):
                            out.append(
                                {
                                    "debug": inst.get("debug", 0),
                                    "engine": inst["engine"],
                                    "ins": [],
                                    "name": f"{inst['name']}_w{k}",
                                    "opcode": "NoOp",
                                    "outs": [],
                                    "sync_info": {"on_update": [], "on_wait": [w]},
                                }
                            )
                        si["on_wait"] = [waits[-1]]
                    out.append(inst)
                b["instructions"] = out
        return m

    def patched(self):
        raw = orig(self)
        try:
            return json.dumps(_split(json.loads(raw))).encode()
        except Exception:
            return raw

    bass.Bass.to_json_bytes = patched
    bass.Bass._multiwait_fix_installed = True


def _install_drain_fix(tile, ScopedClock, VectorClock):
    if getattr(tile.TileContext, "_drain_fix_installed", False):
        return

    def _drain_and_barrier(self, tick_clock, wait_clock):
        gc = tick_clock.global_clock
        n = len(gc)
        for p in range(n):
            t = gc[p]
            if t > 0:
                vc = VectorClock([0] * n)
                vc.require_at_least(p, t)
                d = self.nc.sync.drain()
                wait_clock.add_sem_waits(d.ins, ScopedClock({None: vc}))
        self.nc.all_engine_barrier()
        popped = self.nc._tile_sem_poison_stack.pop()
        assert popped is self._sem_poison
        self.nc.clear_and_free_semaphores(list(self.sems.allocated().values()))
        self.nc.all_engine_barrier()

    tile.TileContext._drain_and_barrier = _drain_and_barrier
    tile.TileContext._drain_fix_installed = True


# ---------------------------------------------------------------------------
# device program
# ---------------------------------------------------------------------------
def _build_nc():
    import concourse.bass as bass
    import concourse.mybir as mybir
    import concourse.tile as tile
    from concourse.vector_clock import ScopedClock, VectorClock

    _install_multiwait_fix(bass)
    _install_drain_fix(tile, ScopedClock, VectorClock)

    dt = mybir.dt
    AF = mybir.ActivationFunctionType
    MUL = mybir.AluOpType.mult
    ADD = mybir.AluOpType.add

    nc = bass.Bass()

    # register EPS so float bias=EPS works on the scalar engine
    _eps_t = nc.alloc_sbuf_tensor(f"const-float32-{EPS}", [128, 1], dt.float32)
    nc.gpsimd.memset(_eps_t.ap(), EPS)
    nc.const_aps.aps[(dt.float32, EPS)] = _eps_t.ap()
    nc.all_engine_barrier()

    # ---- inputs ----
    hTb = nc.dram_tensor("hTb", [H, S], dt.bfloat16, kind="ExternalInput")
    hTqb = nc.dram_tensor("hTqb", [H, QR], dt.bfloat16, kind="ExternalInput")
    hTq = nc.dram_tensor("hTq", [H, QR], dt.float32, kind="ExternalInput")
    # full-seq rope tables for k_pe: [64,S], halves stacked; sin has the
    # rotate-half sign pattern folded in (rows 0:32 = -sin, rows 32:64 = +sin)
    cosT2 = nc.dram_tensor("cosT2", [2 * 32, S], dt.float32, kind="ExternalInput")
    sinT2 = nc.dram_tensor("sinT2", [2 * 32, S], dt.float32, kind="ExternalInput")
    # q-side tables tiled 4x for head-pair rope: [128, QR]
    cosTq4 = nc.dram_tensor("cosTq4", [128, QR], dt.float32, kind="ExternalInput")
    sinTq4 = nc.dram_tensor("sinTq4", [128, QR], dt.float32, kind="ExternalInput")
    masks = nc.dram_tensor("masks", [P, TK, QH], dt.bfloat16, kind="ExternalInput")
    w_qa = nc.dram_tensor("w_qa", [KI_QL, P, KI_H, P], dt.bfloat16, kind="ExternalInput")
    w_qb = nc.dram_tensor("w_qb", [NH, P, KI_QL, NOPE], dt.bfloat16, kind="ExternalInput")
    w_qbr = nc.dram_tensor("w_qbr", [NH // 2, P, KI_QL, 128], dt.bfloat16, kind="ExternalInput")
    w_kva = nc.dram_tensor("w_kva", [P, KI_H, KV_LORA + ROPE], dt.bfloat16, kind="ExternalInput")
    w_kv_k = nc.dram_tensor("w_kv_k", [NH // 4, P, KI_KVL, 512], dt.bfloat16, kind="ExternalInput")
    w_kv_v = nc.dram_tensor("w_kv_v", [NH // 4, P, KI_KVL, 512], dt.bfloat16, kind="ExternalInput")
    w_o = nc.dram_tensor("w_o", [KI_H, P, NH, VHD], dt.bfloat16, kind="ExternalInput")
    w_g = nc.dram_tensor("w_g", [NF_FF, P, KI_H, P], dt.bfloat16, kind="ExternalInput")
    w_u = nc.dram_tensor("w_u", [NF_FF, P, KI_H, P], dt.bfloat16, kind="ExternalInput")
    w_d = nc.dram_tensor("w_d", [KI_H, P, NF_FF, P], dt.bfloat16, kind="ExternalInput")
    out = nc.dram_tensor("out", [H, QR], dt.float32, kind="ExternalOutput")
    h1d = nc.dram_tensor("h1d", [H, QR], dt.float32)  # internal scratch

    def rsqrt_stat(tmp, acc, denom):
        # 1/sqrt(mean + eps) from a [1, N] PSUM sum-of-squares accumulator
        s = tmp.tile([1, acc.shape[-1]], dt.float32, tag="stat", bufs=2)
        nc.scalar.activation(out=s[:], in_=acc[:], func=AF.Sqrt, bias=EPS, scale=1.0 / denom)
        nc.vector.reciprocal(s[:], s[:])
        return s

    import contextlib

    with tile.TileContext(nc) as tc, contextlib.ExitStack() as top:
        tp = lambda **kw: top.enter_context(tc.tile_pool(**kw))
        ones = tp(name="ones", bufs=1)
        tmp = tp(name="tmp", bufs=3)
        ld = tp(name="ld", bufs=3)
        ps = tp(name="ps", bufs=3, space="PSUM")
        ps_acc = tp(name="ps_acc", bufs=1, space="PSUM")
        # attn survives phase 3 -> phase 4; keep at top level (LIFO)
        attn_pool = tp(name="attn_pool", bufs=1)
        attn = attn_pool.tile([P, NH, QR], dt.bfloat16)
        wo_pool = tp(name="wo_pool", bufs=2)

        # [1, P] row for partition replication (K=1 matmul),
        # [P, 1] column for cross-partition reduction (M=1 matmul).
        ones_bf = ones.tile([P, 1], dt.bfloat16)
        nc.vector.memset(ones_bf[:], 1.0)
        ones_f32 = ones.tile([1, P], dt.float32)
        nc.vector.memset(ones_f32[:], 1.0)

        with contextlib.ExitStack() as mid:
            lat = mid.enter_context(tc.tile_pool(name="lat", bufs=1))
            ckv = lat.tile([P, KI_KVL, S], dt.bfloat16)  # normalized kv latents
            kpe2 = lat.tile([P, S], dt.bfloat16)  # roped shared key-pe, stacked twice
            pA = mid.enter_context(tc.tile_pool(name="pA", bufs=1))
            xqbf = pA.tile([P, KI_H, QR], dt.bfloat16)
            s1qrep = pA.tile([P, QR], dt.float32)

            # ==== phase 0+1: ln1 stats + kv latents (per 512-column chunk) ====
            with tc.tile_pool(name="pB", bufs=1) as pB:
                wkva = pB.tile([P, KI_H, KV_LORA + ROPE], dt.bfloat16)
                nc.sync.dma_start(wkva[:], w_kva[:])
                cosb = pB.tile([2 * 32, S], dt.float32)
                sinb = pB.tile([2 * 32, S], dt.float32)
                nc.sync.dma_start(cosb[:], cosT2[:])
                nc.sync.dma_start(sinb[:], sinT2[:])

                # q-slice ln1 stats + bf16 cast
                accq = ps_acc.tile([1, QR], dt.float32, tag="acc")
                for ki in range(KI_H):
                    nc.sync.dma_start(xqbf[:, ki, :], hTqb[ki * P : (ki + 1) * P, :])
                    sq = tmp.tile([P, QR], dt.bfloat16, tag="sq")
                    nc.vector.tensor_tensor(sq[:], xqbf[:, ki, :], xqbf[:, ki, :], MUL)
                    nc.tensor.matmul(
                        accq[:], ones_bf[:], sq[:], start=(ki == 0), stop=(ki == KI_H - 1)
                    )
                s1q = rsqrt_stat(tmp, accq, H)
                repq = ps.tile([P, QR], dt.float32, tag="mm")
                nc.tensor.matmul(repq[:], ones_f32[:], s1q[:], start=True, stop=True)
                nc.vector.tensor_copy(s1qrep[:], repq[:])

                # kv_a rmsnorm of chunk t is deferred into chunk t+1 so the
                # PE never waits on the sqrt/reciprocal chain
                pending_kvnorm = [None]

                for t in range(S // 512):
                    tsl = slice(t * 512, (t + 1) * 512)
                    xc = pB.tile([P, KI_H, 512], dt.bfloat16, tag="xc", bufs=2)
                    acc = ps_acc.tile([1, 512], dt.float32, tag="acc")
                    for ki in range(KI_H):
                        nc.sync.dma_start(xc[:, ki, :], hTb[ki * P : (ki + 1) * P, tsl])
                        sq = tmp.tile([P, 512], dt.bfloat16, tag="sq")
                        nc.vector.tensor_tensor(sq[:], xc[:, ki, :], xc[:, ki, :], MUL)
                        nc.tensor.matmul(
                            acc[:], ones_bf[:], sq[:], start=(ki == 0), stop=(ki == KI_H - 1)
                        )
                    if pending_kvnorm[0] is not None:
                        pending_kvnorm[0]()
                        pending_kvnorm[0] = None
                    s1 = rsqrt_stat(tmp, acc, H)
                    s1r = tmp.tile([P, 512], dt.float32, tag="s1r", bufs=2)

                    kvacc = ps_acc.tile([1, 512], dt.float32, tag="acc")
                    rep1_done = False
                    sqs = []
                    for nf in range(KI_KVL):
                        pt = ps.tile([P, 512], dt.float32, tag="mm")
                        for ki in range(KI_H):
                            nc.tensor.matmul(
                                pt[:],
                                wkva[:, ki, nf * P : (nf + 1) * P],
                                xc[:, ki, :],
                                start=(ki == 0),
                                stop=(ki == KI_H - 1),
                            )
                        if not rep1_done:
                            # replicate the ln1 scale while the PE is busy
                            rep1 = ps.tile([P, 512], dt.float32, tag="mm")
                            nc.tensor.matmul(rep1[:], ones_f32[:], s1[:], start=True, stop=True)
                            nc.vector.tensor_copy(s1r[:], rep1[:])
                            rep1_done = True
                        # ln1 1/rms column scale applied on the way out of PSUM
                        nc.vector.tensor_tensor(ckv[:, nf, tsl], pt[:], s1r[:], MUL)
                        sq = tmp.tile([P, 512], dt.bfloat16, tag="sqkv", bufs=4)
                        nc.vector.tensor_tensor(sq[:], ckv[:, nf, tsl], ckv[:, nf, tsl], MUL)
                        sqs.append(sq)
                    # k_pe: last 64 cols of w_kva, ln1-scaled, then RoPE
                    pt = ps.tile([ROPE, 512], dt.float32, tag="mm")
                    for ki in range(KI_H):
                        nc.tensor.matmul(
                            pt[:],
                            wkva[:, ki, KV_LORA : KV_LORA + ROPE],
                            xc[:, ki, :],
                            start=(ki == 0),
                            stop=(ki == KI_H - 1),
                        )
                    # kv_a sum-of-squares, batched here so the matmuls never
                    # head-of-line block the PE queue on the exit/recip chain
                    for nf in range(KI_KVL):
                        nc.tensor.matmul(
                            kvacc[:], ones_bf[:], sqs[nf][:],
                            start=(nf == 0), stop=(nf == KI_KVL - 1),
                        )
                    pes = tmp.tile([ROPE, 512], dt.float32, tag="pes", bufs=2)
                    nc.vector.tensor_tensor(pes[:], pt[:], s1r[:ROPE, :], MUL)
                    # swap halves so rope is 3 full-width DVE ops
                    xsw = tmp.tile([ROPE, 512], dt.float32, tag="xsw", bufs=2)
                    nc.sync.dma_start(xsw[:32, :], pes[32:, :])
                    nc.sync.dma_start(xsw[32:, :], pes[:32, :])
                    m1 = tmp.tile([ROPE, 512], dt.float32, tag="t1", bufs=2)
                    m2 = tmp.tile([ROPE, 512], dt.float32, tag="t2", bufs=2)
                    nc.vector.tensor_tensor(m1[:], pes[:], cosb[:, tsl], MUL)
                    nc.vector.tensor_tensor(m2[:], xsw[:], sinb[:, tsl], MUL)
                    nc.vector.tensor_tensor(kpe2[:ROPE, tsl], m1[:], m2[:], ADD)
                    nc.sync.dma_start(kpe2[ROPE:, tsl], kpe2[:ROPE, tsl])

                    def _kvnorm(tsl=tsl, kvacc=kvacc):
                        skv = rsqrt_stat(tmp, kvacc, KV_LORA)
                        repkv = ps.tile([P, 512], dt.float32, tag="mm")
                        nc.tensor.matmul(repkv[:], ones_f32[:], skv[:], start=True, stop=True)
                        rkv = tmp.tile([P, 512], dt.float32, tag="s1r", bufs=2)
                        nc.vector.tensor_copy(rkv[:], repkv[:])
                        for nf in range(KI_KVL):
                            nc.vector.tensor_tensor(ckv[:, nf, tsl], ckv[:, nf, tsl], rkv[:], MUL)

                    pending_kvnorm[0] = _kvnorm
                if pending_kvnorm[0] is not None:
                    pending_kvnorm[0]()
                    pending_kvnorm[0] = None

            # ==== phase 2: q path ====
            with contextlib.ExitStack() as sc2:
                qnp = sc2.enter_context(tc.tile_pool(name="qnp", bufs=1))
                qn = qnp.tile([P, NH, QR], dt.bfloat16)  # q nope (fm)
                # q pe roped, head pairs stacked: rows 0:64 head 2p, 64:128 head 2p+1
                qpp = qnp.tile([P, NH // 2, QR], dt.bfloat16)
                maskt = qnp.tile([P, TK, QH], dt.bfloat16)
                nc.sync.dma_start(maskt[:], masks[:])
                with tc.tile_pool(name="p2", bufs=1) as p2:
                    qlat = p2.tile([P, KI_QL, QR], dt.bfloat16)
                    qacc = ps_acc.tile([1, QR], dt.float32, tag="acc")
                    for nf in range(KI_QL):
                        wt = p2.tile([P, KI_H, P], dt.bfloat16, tag="wqa", bufs=2)
                        nc.sync.dma_start(wt[:], w_qa[nf])
                        pt = ps.tile([P, QR], dt.float32, tag="mm")
                        for ki in range(KI_H):
                            nc.tensor.matmul(
                                pt[:],
                                wt[:, ki, :],
                                xqbf[:, ki, :],
                                start=(ki == 0),
                                stop=(ki == KI_H - 1),
                            )
                        nc.vector.tensor_tensor(qlat[:, nf, :], pt[:], s1qrep[:], MUL)
                        sq = tmp.tile([P, QR], dt.bfloat16, tag="sq")
                        nc.vector.tensor_tensor(sq[:], qlat[:, nf, :], qlat[:, nf, :], MUL)
                        nc.tensor.matmul(
                            qacc[:], ones_bf[:], sq[:], start=(nf == 0), stop=(nf == KI_QL - 1)
                        )
                    sqv = rsqrt_stat(tmp, qacc, Q_LORA)
                    sqrep = p2.tile([P, QR], dt.float32)

                    # 4x-tiled rope tables for q with the q_a_ln scale folded in
                    cosq = p2.tile([P, QR], dt.float32)
                    sinq = p2.tile([P, QR], dt.float32)
                    nc.sync.dma_start(cosq[:], cosTq4[:])
                    nc.sync.dma_start(sinq[:], sinTq4[:])

                    for h in range(NH):
                        wt = p2.tile([P, KI_QL, NOPE], dt.bfloat16, tag="wqb", bufs=2)
                        nc.sync.dma_start(wt[:], w_qb[h])
                        pt = ps.tile([P, QR], dt.float32, tag="mm")
                        for ki in range(KI_QL):
                            nc.tensor.matmul(
                                pt[:],
                                wt[:, ki, :],
                                qlat[:, ki, :],
                                start=(ki == 0),
                                stop=(ki == KI_QL - 1),
                            )
                        if h == 0:
                            # replicate the q_a rmsnorm scale behind head 0's matmuls
                            repsq = ps.tile([P, QR], dt.float32, tag="mm")
                            nc.tensor.matmul(repsq[:], ones_f32[:], sqv[:], start=True, stop=True)
                            nc.vector.tensor_copy(sqrep[:], repsq[:])
                            nc.vector.tensor_tensor(cosq[:], cosq[:], sqrep[:], MUL)
                            nc.vector.tensor_tensor(sinq[:], sinq[:], sqrep[:], MUL)
                        nc.vector.tensor_tensor(qn[:, h, :], pt[:], sqrep[:], MUL)

                    for hp in range(NH // 2):
                        wtr = p2.tile([P, KI_QL, 128], dt.bfloat16, tag="wqbr", bufs=2)
                        nc.sync.dma_start(wtr[:], w_qbr[hp])
                        pt2 = ps.tile([P, QR], dt.float32, tag="mm")
                        for ki in range(KI_QL):
                            nc.tensor.matmul(
                                pt2[:],
                                wtr[:, ki, :],
                                qlat[:, ki, :],
                                start=(ki == 0),
                                stop=(ki == KI_QL - 1),
                            )
                        pesq = tmp.tile([P, QR], dt.float32, tag="pes", bufs=2)
                        nc.vector.tensor_copy(pesq[:], pt2[:])
                        xswq = tmp.tile([P, QR], dt.float32, tag="xsw", bufs=2)
                        nc.sync.dma_start(xswq[:32, :], pesq[32:64, :])
                        nc.sync.dma_start(xswq[32:64, :], pesq[:32, :])
                        nc.sync.dma_start(xswq[64:96, :], pesq[96:, :])
                        nc.sync.dma_start(xswq[96:, :], pesq[64:96, :])
                        m1 = tmp.tile([P, QR], dt.float32, tag="t1", bufs=2)
                        m2 = tmp.tile([P, QR], dt.float32, tag="t2", bufs=2)
                        nc.vector.tensor_tensor(m1[:], pesq[:], cosq[:], MUL)
                        nc.vector.tensor_tensor(m2[:], xswq[:], sinq[:], MUL)
                        nc.vector.tensor_tensor(qpp[:, hp, :], m1[:], m2[:], ADD)

                # ==== phase 3: attention ====
                # kt < 8: both strips (width QR); kt >= 8: late strip only
                # (width QH, query cols QH:QR). Masks: E mask for kt<8
                # (L half is fully visible there), L mask for kt>=8.
                with tc.tile_pool(name="p3", bufs=1) as p3:
                    # prefetch o_proj's first weight tile so phase 4 starts hot
                    wo0 = wo_pool.tile([P, NH, VHD], dt.bfloat16, tag="wo")
                    nc.sync.dma_start(wo0[:], w_o[0])
                    wkh_tiles = {}
                    wvh_tiles = {}

                    def get_wkh(hg):
                        if hg not in wkh_tiles:
                            wkh = p3.tile([P, KI_KVL, 512], dt.bfloat16, tag="wkh", bufs=2)
                            nc.sync.dma_start(wkh[:], w_kv_k[hg])
                            wkh_tiles[hg] = wkh
                        return wkh_tiles[hg]

                    def get_wvh(hg):
                        if hg not in wvh_tiles:
                            wvh = p3.tile([P, KI_KVL, 512], dt.bfloat16, tag="wvh", bufs=2)
                            nc.sync.dma_start(wvh[:], w_kv_v[hg])
                            wvh_tiles[hg] = wvh
                        return wvh_tiles[hg]

                    def build_k(h):
                        # k_nope for head h, feature-major [NOPE, S]
                        hg, hh = h // 4, h % 4
                        wkh = get_wkh(hg)
                        ksb = p3.tile([P, S], dt.bfloat16, tag="ksb", bufs=3)
                        for t in range(S // 512):
                            pt = ps.tile([P, 512], dt.float32, tag="mm")
                            for lt in range(KI_KVL):
                                nc.tensor.matmul(
                                    pt[:],
                                    wkh[:, lt, hh * P : (hh + 1) * P],
                                    ckv[:, lt, t * 512 : (t + 1) * 512],
                                    start=(lt == 0),
                                    stop=(lt == KI_KVL - 1),
                                )
                            nc.vector.tensor_copy(ksb[:, t * 512 : (t + 1) * 512], pt[:])
                        return ksb

                    ksbs = {0: build_k(0), 1: build_k(1)}
                    pending_attn = [None]
                    for hg in range(NH // 4):
                        wvh = get_wvh(hg)
                        # v for 4 heads at once: v_rm[kpos, 4*VHD]
                        vsb = p3.tile([P, TK, 4 * VHD], dt.bfloat16, tag="vsb", bufs=1)
                        for kt in range(TK):
                            pt = ps.tile([P, 4 * VHD], dt.float32, tag="mm")
                            for lt in range(KI_KVL):
                                nc.tensor.matmul(
                                    pt[:],
                                    ckv[:, lt, kt * P : (kt + 1) * P],
                                    wvh[:, lt, :],
                                    start=(lt == 0),
                                    stop=(lt == KI_KVL - 1),
                                )
                            nc.vector.tensor_copy(vsb[:, kt, :], pt[:])
                        for hh in range(4):
                            h = hg * 4 + hh
                            if hh == 2:
                                get_wvh(hg + 1) if hg + 1 < NH // 4 else None
                            ksb = ksbs.pop(h)
                            qoff = (h % 2) * 64
                            # scores / masked exp / attnV over all key tiles
                            av = ps_acc.tile([P, QR], dt.float32, tag="av", bufs=2)
                            se = ps_acc.tile([1, QR], dt.float32, tag="se", bufs=2)
                            # 2-deep software pipeline: emit se/av for kt-2 so
                            # the PE never stalls on the exp+mask chain.
                            DELAY = 2
                            prs = {}

                            def _drain_kt(kt, av=av, se=se, vsb=vsb, hh=hh):
                                pr, csl = prs.pop(kt)
                                nc.tensor.matmul(
                                    se[:, csl], ones_bf[:], pr[:],
                                    start=(kt == 0), stop=(kt == TK - 1),
                                    skip_group_check=True,
                                )
                                nc.tensor.matmul(
                                    av[:, csl], vsb[:, kt, hh * VHD : (hh + 1) * VHD], pr[:],
                                    start=(kt == 0), stop=(kt == TK - 1),
                                    skip_group_check=True,
                                )

                            for kt in range(TK):
                                wide = kt < 8
                                csl = slice(0, QR) if wide else slice(QH, QR)
                                n = QR if wide else QH
                                sc = ps.tile([P, QR], dt.float32, tag="mm")
                                nc.tensor.matmul(
                                    sc[:, :n], ksb[:, kt * P : (kt + 1) * P], qn[:, h, csl],
                                    start=True, stop=False,
                                )
                                nc.tensor.matmul(
                                    sc[:, :n],
                                    kpe2[qoff : qoff + 64, kt * P : (kt + 1) * P],
                                    qpp[qoff : qoff + 64, h // 2, csl],
                                    start=False, stop=True,
                                )
                                pr = tmp.tile([P, n], dt.bfloat16, tag="pr" if wide else "prn", bufs=4)
                                nc.scalar.activation(
                                    out=pr[:], in_=sc[:, :n], func=AF.Exp, scale=ATTN_SCALE
                                )
                                # kt<8: mask only the early strip (cols :QH);
                                # kt>=8: mask the late strip (all cols of pr).
                                nc.vector.tensor_tensor(
                                    pr[:, :QH], pr[:, :QH], maskt[:, kt, :], MUL
                                )
                                prs[kt] = (pr, csl)
                                if kt == 6 and pending_attn[0] is not None:
                                    # previous head's softmax normalization:
                                    # by now its 3.3us reciprocal has finished,
                                    # so the replication matmul won't stall the
                                    # PE queue
                                    pending_attn[0]()
                                    pending_attn[0] = None
                                if kt >= DELAY:
                                    _drain_kt(kt - DELAY)
                            # overlap a later head's k_nope build with this
                            # head's softmax/AV tail (depth-2 pipeline)
                            if h + 2 < NH:
                                ksbs[h + 2] = build_k(h + 2)
                            for kt in range(TK - DELAY, TK):
                                _drain_kt(kt)
                            # start the reciprocal now (DVE), but defer the
                            # replication matmul + attn write into the next
                            # head's score loop
                            rc = tmp.tile([1, QR], dt.float32, tag="stat", bufs=2)
                            nc.vector.reciprocal(rc[:], se[:])

                            def _attn_norm(av=av, rc=rc, h=h):
                                repr_ = ps.tile([P, QR], dt.float32, tag="mm")
                                nc.tensor.matmul(repr_[:], ones_f32[:], rc[:], start=True, stop=True)
                                rsb = tmp.tile([P, QR], dt.float32, tag="s1r", bufs=2)
                                nc.vector.tensor_copy(rsb[:], repr_[:])
                                nc.vector.tensor_tensor(attn[:, h, :], av[:], rsb[:], MUL)

                            pending_attn[0] = _attn_norm
                    pending_attn[0]()
                    pending_attn[0] = None

        # ==== phase 4: o_proj + residual + ln2 (h1 staged via DRAM) ====
        with contextlib.ExitStack() as sc45:
            x2m = sc45.enter_context(tc.tile_pool(name="x2m", bufs=1))
            x2 = x2m.tile([P, KI_H, QR], dt.bfloat16)
            msb = x2m.tile([P, NF_FF, QR], dt.bfloat16)
            s2rep = x2m.tile([P, QR], dt.float32)
            wg0 = x2m.tile([P, KI_H, P], dt.bfloat16)
            nc.sync.dma_start(wg0[:], w_g[0])
            wu0 = x2m.tile([P, KI_H, P], dt.bfloat16)
            nc.sync.dma_start(wu0[:], w_u[0])
            with tc.tile_pool(name="p4", bufs=1) as p4:
                oacc = ps_acc.tile([1, QR], dt.float32, tag="acc")
                for nf in range(KI_H):
                    if nf == 0:
                        wt = wo0
                    else:
                        wt = wo_pool.tile([P, NH, VHD], dt.bfloat16, tag="wo")
                        nc.sync.dma_start(wt[:], w_o[nf])
                    pt = ps.tile([P, QR], dt.float32, tag="mm")
                    for kh in range(NH):
                        nc.tensor.matmul(
                            pt[:],
                            wt[:, kh, :],
                            attn[:, kh, :],
                            start=(kh == 0),
                            stop=(kh == NH - 1),
                        )
                    ht = ld.tile([P, QR], dt.float32, tag="hload")
                    nc.sync.dma_start(ht[:], hTq[nf * P : (nf + 1) * P, :])
                    h1t = tmp.tile([P, QR], dt.float32, tag="h1t", bufs=2)
                    nc.vector.tensor_tensor(h1t[:], pt[:], ht[:], ADD)
                    nc.sync.dma_start(h1d[nf * P : (nf + 1) * P, :], h1t[:])
                    # x2 holds UNNORMALIZED h1 (bf16); the ln2 1/rms scale is
                    # per-token (per matmul column) so it commutes through the
                    # FFN GEMMs and is applied at the gate/down exits instead.
                    # This unblocks the FFN from the ln2 stats chain.
                    nc.vector.tensor_copy(x2[:, nf, :], h1t[:])
                    sq = tmp.tile([P, QR], dt.bfloat16, tag="sq")
                    nc.vector.tensor_tensor(sq[:], h1t[:], h1t[:], MUL)
                    nc.tensor.matmul(
                        oacc[:], ones_bf[:], sq[:], start=(nf == 0), stop=(nf == KI_H - 1)
                    )
                s2 = rsqrt_stat(tmp, oacc, H)
                reps2 = ps.tile([P, QR], dt.float32, tag="mm")
                nc.tensor.matmul(reps2[:], ones_f32[:], s2[:], start=True, stop=True)
                nc.vector.tensor_copy(s2rep[:], reps2[:])

            # ==== phase 5: FFN (SwiGLU) ====
            with tc.tile_pool(name="p5", bufs=1) as p5:
                for nf in range(NF_FF):
                    if nf == 0:
                        wtg = wg0
                    else:
                        wtg = p5.tile([P, KI_H, P], dt.bfloat16, tag="wg", bufs=2)
                        nc.sync.dma_start(wtg[:], w_g[nf])
                    pg = ps.tile([P, QR], dt.float32, tag="mm")
                    for ki in range(KI_H):
                        nc.tensor.matmul(
                            pg[:], wtg[:, ki, :], x2[:, ki, :],
                            start=(ki == 0), stop=(ki == KI_H - 1),
                        )
                    gsc = tmp.tile([P, QR], dt.float32, tag="h1t", bufs=2)
                    nc.vector.tensor_tensor(gsc[:], pg[:], s2rep[:], MUL)
                    gs = tmp.tile([P, QR], dt.bfloat16, tag="sq")
                    nc.scalar.activation(out=gs[:], in_=gsc[:], func=AF.Silu)
                    if nf == 0:
                        wtu = wu0
                    else:
                        wtu = p5.tile([P, KI_H, P], dt.bfloat16, tag="wu", bufs=2)
                        nc.sync.dma_start(wtu[:], w_u[nf])
                    pu = ps.tile([P, QR], dt.float32, tag="mm")
                    for ki in range(KI_H):
                        nc.tensor.matmul(
                            pu[:], wtu[:, ki, :], x2[:, ki, :],
                            start=(ki == 0), stop=(ki == KI_H - 1),
                        )
                    nc.vector.tensor_tensor(msb[:, nf, :], pu[:], gs[:], MUL)

                for nf in range(KI_H):
                    pt = ps.tile([P, QR], dt.float32, tag="mm")
                    for half in range(2):
                        wt = p5.tile([P, NF_FF // 2, P], dt.bfloat16, tag="wd", bufs=2)
                        nc.sync.dma_start(wt[:], w_d[nf, :, half * 32 : (half + 1) * 32, :])
                        for ki in range(NF_FF // 2):
                            kk = half * 32 + ki
                            nc.tensor.matmul(
                                pt[:], wt[:, ki, :], msb[:, kk, :],
                                start=(kk == 0), stop=(kk == NF_FF - 1),
                            )
                    hb = ld.tile([P, QR], dt.float32, tag="hload")
                    nc.sync.dma_start(hb[:], h1d[nf * P : (nf + 1) * P, :])
                    psc = tmp.tile([P, QR], dt.float32, tag="psc", bufs=2)
                    nc.vector.tensor_tensor(psc[:], pt[:], s2rep[:], MUL)
                    ot = tmp.tile([P, QR], dt.float32, tag="h1t", bufs=2)
                    nc.vector.tensor_tensor(ot[:], psc[:], hb[:], ADD)
                    nc.sync.dma_start(out[nf * P : (nf + 1) * P, :], ot[:])

    return nc


# ---------------------------------------------------------------------------
# host-side packing
# ---------------------------------------------------------------------------
def _deint_perm():
    # deinterleave: out[i] = in[2i] (i<32), in[2(i-32)+1] (i>=32)
    return np.concatenate([np.arange(0, ROPE, 2), np.arange(1, ROPE, 2)])


def _pack_lhst(w, nki, nnf, nfree=P):
    # w [nki*P, nnf*nfree] -> [nnf, P, nki, nfree]
    return np.ascontiguousarray(
        w.reshape(nki, P, nnf, nfree).transpose(2, 1, 0, 3).astype(BF16)
    )


def _qcols(j):
    # query columns of core (b, j): early strip then late strip
    return np.concatenate(
        [np.arange(j * QH, (j + 1) * QH), np.arange((7 - j) * QH, (8 - j) * QH)]
    )


def _prep_shared(inputs):
    perm = _deint_perm()
    ln1 = inputs["ln1_w"].astype(np.float32)
    qaln = inputs["q_a_ln_w"].astype(np.float32)
    kvln = inputs["kv_a_ln_w"].astype(np.float32)
    ln2 = inputs["ln2_w"].astype(np.float32)

    w_qa = inputs["q_a_kernel"].astype(np.float32) * ln1[:, None]
    w_kva = inputs["kv_a_kernel"].astype(np.float32) * ln1[:, None]
    w_kva = w_kva.copy()
    w_kva[:, KV_LORA:] = w_kva[:, KV_LORA:][:, perm]
    w_qb = inputs["q_b_kernel"].astype(np.float32) * qaln[:, None]
    # split nope / deinterleaved rope columns per head
    w_qb_nope = np.empty((Q_LORA, NH, NOPE), np.float32)
    w_qb_rope = np.empty((Q_LORA, NH, ROPE), np.float32)
    for h in range(NH):
        blk = w_qb[:, h * QHD : (h + 1) * QHD]
        w_qb_nope[:, h] = blk[:, :NOPE]
        w_qb_rope[:, h] = blk[:, NOPE:][:, perm]
    # head-pair rope weights: cols [h2p half1 | h2p half2 | h2p+1 half1 | h2p+1 half2]
    w_qbr = np.empty((Q_LORA, NH // 2, 128), np.float32)
    for hp in range(NH // 2):
        w_qbr[:, hp, 0:64] = w_qb_rope[:, 2 * hp]
        w_qbr[:, hp, 64:128] = w_qb_rope[:, 2 * hp + 1]
    w_kvb = inputs["kv_b_kernel"].astype(np.float32) * kvln[:, None]
    w_o = inputs["o_kernel"].astype(np.float32)
    w_g = inputs["gate_kernel"].astype(np.float32) * ln2[:, None]
    w_u = inputs["up_kernel"].astype(np.float32) * ln2[:, None]
    w_d = inputs["down_kernel"].astype(np.float32)

    shared = {
        "w_qa": _pack_lhst(w_qa, KI_H, KI_QL),
        # w_qb: [NH, P, KI_QL, NOPE]
        "w_qb": np.ascontiguousarray(
            w_qb_nope.reshape(KI_QL, P, NH, NOPE).transpose(2, 1, 0, 3).astype(BF16)
        ),
        # w_qbr: [NH/2, P, KI_QL, 128]
        "w_qbr": np.ascontiguousarray(
            w_qbr.reshape(KI_QL, P, NH // 2, 128).transpose(2, 1, 0, 3).astype(BF16)
        ),
        # w_kva resident: [P, KI_H, 576]
        "w_kva": np.ascontiguousarray(
            w_kva.reshape(KI_H, P, KV_LORA + ROPE).transpose(1, 0, 2).astype(BF16)
        ),
        # w_kvb split into k/v halves, packed per head-group of 4:
        # [hg, p, lt, hh*128+c]
        "w_kv_k": np.ascontiguousarray(
            w_kvb.reshape(KI_KVL, P, NH // 4, 4, 2, 128)[:, :, :, :, 0, :]
            .transpose(2, 1, 0, 3, 4)
            .reshape(NH // 4, P, KI_KVL, 512)
            .astype(BF16)
        ),
        "w_kv_v": np.ascontiguousarray(
            w_kvb.reshape(KI_KVL, P, NH // 4, 4, 2, 128)[:, :, :, :, 1, :]
            .transpose(2, 1, 0, 3, 4)
            .reshape(NH // 4, P, KI_KVL, 512)
            .astype(BF16)
        ),
        # w_o: [KI_H(nf), P, NH, VHD]
        "w_o": np.ascontiguousarray(
            w_o.reshape(NH, VHD, KI_H, P).transpose(2, 1, 0, 3).astype(BF16)
        ),
        "w_g": _pack_lhst(w_g, KI_H, NF_FF),
        "w_u": _pack_lhst(w_u, KI_H, NF_FF),
        "w_d": _pack_lhst(w_d, NF_FF, KI_H),
    }
    return shared


def _prep_batch(inputs, b):
    hid = np.asarray(inputs["hidden_states"][b], dtype=np.float32)  # [S, H]
    hT = np.ascontiguousarray(hid.T)  # [H, S]
    pos = np.asarray(inputs["position_ids"][b]).astype(np.int64)
    cos_g = np.asarray(inputs["cos"], dtype=np.float32)[pos][:, :32]  # [S, 32]
    sin_g = np.asarray(inputs["sin"], dtype=np.float32)[pos][:, :32]
    return hT, np.ascontiguousarray(cos_g.T), np.ascontiguousarray(sin_g.T)


def _core_masks(j):
    # [P, TK, QH]: kt<8 -> early-strip mask, kt>=8 -> late-strip mask
    kp = np.arange(P)[:, None]
    qf = np.arange(QH)[None, :]
    m = np.zeros((P, TK, QH), dtype=BF16)
    for kt in range(TK):
        q0 = j * QH if kt < 8 else (7 - j) * QH
        m[:, kt, :] = ((kt * P + kp) <= (q0 + qf)).astype(BF16)
    return m


def kernel(**inputs) -> np.ndarray:
    import concourse.bass as bass  # noqa: F401  (env check)
    from concourse.bass_utils import run_bass_kernel_spmd

    if "nc" not in _COMPILED:
        _COMPILED["nc"] = _build_nc()
    nc = _COMPILED["nc"]

    shared = _prep_shared(inputs)
    in_maps = []
    per_batch = [_prep_batch(inputs, b) for b in range(B)]
    hTb_cache = {}
    for c in range(8):
        b, j = c // 4, c % 4
        hT, cosT, sinT = per_batch[b]
        if b not in hTb_cache:
            hTb_cache[b] = hT.astype(BF16)
        hTb = hTb_cache[b]
        qc = _qcols(j)
        # k_pe rope tables: halves stacked, sign folded into sin
        cosT2 = np.concatenate([cosT, cosT], axis=0)
        sinT2 = np.concatenate([-sinT, sinT], axis=0)
        # q rope tables for this core's columns, tiled 4x (2 heads x 2 halves)
        cq = cosT[:, qc]
        sq = sinT[:, qc]
        in_map = dict(shared)
        in_map["hTb"] = hTb
        in_map["hTqb"] = np.ascontiguousarray(hTb[:, qc])
        in_map["hTq"] = np.ascontiguousarray(hT[:, qc])
        in_map["cosT2"] = np.ascontiguousarray(cosT2)
        in_map["sinT2"] = np.ascontiguousarray(sinT2)
        in_map["cosTq4"] = np.ascontiguousarray(np.tile(cq, (4, 1)))
        in_map["sinTq4"] = np.ascontiguousarray(np.tile(np.concatenate([-sq, sq], 0), (2, 1)))
        in_map["masks"] = _core_masks(j)
        in_maps.append(in_map)

    res = run_bass_kernel_spmd(nc, in_maps, core_ids=list(range(8)))
    globals()["LAST_RESULT"] = res

    out = np.empty((B, S, H), dtype=np.float32)
    for c in range(8):
        b, j = c // 4, c % 4
        r = res.results[c]["out"]
        out[b, j * QH : (j + 1) * QH, :] = r[:, :QH].T
        out[b, (7 - j) * QH : (8 - j) * QH, :] = r[:, QH:].T
    return out
